# revision 1
# baseline (speedup 1.0000x reference)
"""Expert-parallel MoE (top-1 routing) kernel for 8 TRN2 NeuronCores.

Strategy (per the expert-parallel sharding hint): the 8 experts are sharded
1:1 across the 8 cores. The router is a 0.1%-of-FLOPs linear; it is computed
host-side in float64 to decide the token->expert dispatch (the all-to-all is
realized as the host->device sharding itself: each token's activations are
DMA'd only to the core owning its expert). Each core then runs the dense
expert MLP  y = (silu(x @ gw.T) * (x @ up.T)) @ dw.T  over its gathered
tokens (padded to a uniform capacity C) in bf16 with fp32 PSUM accumulation.

Layout: everything on device is kept "activation-transposed" so all three
matmuls contract over the partition dimension with zero on-device transposes:
  g_T[i_tile] = sum_k gwT[k, i].T @ x_T[k]      (psum [128(I), C])
  a_T = silu(g_T) * u_T                          (sbuf bf16)
  y_T[m_tile] += dwT[i, m].T @ a_T[i]            (psum [128(H), C], 22-step acc)
Weights are pre-transposed + bf16-cast host-side and packed per i-tile
(gate|up|down) so each iteration issues ONE contiguous 768 KiB DMA.

Raw bass (no Tile scheduler): Tile's sem assignment overflows walrus's
per-instruction sync-wait encoding limits on this kernel; explicit
per-engine streams with standalone wait_ge instructions avoid that
entirely and give a deterministic software pipeline:
  SP   : x DMA, 22 weight DMAs, 8 output DMAs (gated on DVE copies)
  PE   : per i: 8 g-matmuls, 8 u-matmuls (gated on w DMA i), 8 y-matmuls
         (gated on a_T[i] from DVE)
  ACT  : per i: silu(g)->sbuf, copy(u)->sbuf   (gated on PE psum stops)
  DVE  : per i: a_T[i] = sg*us (bf16)          (gated on ACT)
         tail: 8 psum->sbuf y copies           (gated on last PE matmul)
"""

import numpy as np
import ml_dtypes
from contextlib import ExitStack

import concourse.bass as bass
import concourse.mybir as mybir
from concourse.bass_utils import run_bass_kernel_spmd

S, B, H, I, E = 512, 2, 1024, 2816, 8
KT, IT, MT = H // 128, I // 128, H // 128  # 8, 22, 8
_BF = mybir.dt.bfloat16
_F32 = mybir.dt.float32

_nc_cache: dict = {}


def _build(C: int) -> bass.Bass:
    """One-core program; SPMD across 8 cores (same shapes, per-core data)."""
    nc = bass.Bass()
    xt = nc.dram_tensor("xt", [128, KT * C], _BF, kind="ExternalInput")
    # packed weights per i-tile: [gate (KT*128) | up (KT*128) | down (MT*128)]
    wt = nc.dram_tensor("wt", [IT, 128, 3 * KT * 128], _BF, kind="ExternalInput")
    yt = nc.dram_tensor("yt", [128, MT * C], _F32, kind="ExternalOutput")

    assert 2 * C <= 512, "two y slices must fit one PSUM bank"
    GW0, UW0, DW0 = 0, KT * 128, 2 * KT * 128
    W = 3 * KT * 128  # 3072 cols per i-tile

    with ExitStack() as ctx:
        x_sb = ctx.enter_context(nc.sbuf_tensor([128, KT * C], _BF))
        w_sb = ctx.enter_context(nc.sbuf_tensor([128, IT * W], _BF))
        sg_sb = ctx.enter_context(nc.sbuf_tensor([128, IT * C], _F32))
        us_sb = ctx.enter_context(nc.sbuf_tensor([128, IT * C], _F32))
        a_sb = ctx.enter_context(nc.sbuf_tensor([128, IT * C], _BF))
        y_sb = ctx.enter_context(nc.sbuf_tensor([128, MT * C], _F32))
        # every PSUM tensor is one full 2 KiB bank ([128, 512] f32): matmul
        # outputs must not cross bank boundaries, and the bump allocator
        # would otherwise pack tensors across banks
        g_ps = [
            ctx.enter_context(nc.psum_tensor(f"g_ps{j}", [128, 512], _F32))
            for j in range(2)
        ]
        u_ps = [
            ctx.enter_context(nc.psum_tensor(f"u_ps{j}", [128, 512], _F32))
            for j in range(2)
        ]
        y_ps = [
            ctx.enter_context(nc.psum_tensor(f"y_ps{j}", [128, 512], _F32))
            for j in range(4)
        ]

        def yslice(m):
            return y_ps[m // 2][:, (m % 2) * 256 : (m % 2) * 256 + C]

        x_sem = ctx.enter_context(nc.semaphore())
        w_sem = [ctx.enter_context(nc.semaphore(name=f"w_sem{j}")) for j in range(IT)]
        pe_g = ctx.enter_context(nc.semaphore())
        pe_u = ctx.enter_context(nc.semaphore())
        pe_done = ctx.enter_context(nc.semaphore())
        act_sem = ctx.enter_context(nc.semaphore())
        dve_sem = ctx.enter_context(nc.semaphore())
        dma_sem = ctx.enter_context(nc.semaphore())
        block = ctx.enter_context(nc.Block())

        @block.sync
        def _(sync):
            nc.sync.dma_start(x_sb[:], xt[:]).then_inc(x_sem, 16)
            for i in range(IT):
                nc.sync.dma_start(
                    w_sb[:, i * W : (i + 1) * W], wt[i]
                ).then_inc(w_sem[i], 16)
            for m in range(MT):
                # copy m done once dve_sem reaches IT (muls) + m+1 (copies)
                nc.sync.wait_ge(dve_sem, IT + m + 1)
                nc.sync.dma_start(
                    yt[:, m * C : (m + 1) * C], y_sb[:, m * C : (m + 1) * C]
                ).then_inc(dma_sem, 16)
            nc.sync.wait_ge(dma_sem, MT * 16)

        @block.tensor
        def _(tensor):
            nc.tensor.wait_ge(x_sem, 16)
            for i in range(IT):
                pp = i % 2
                nc.tensor.wait_ge(w_sem[i], 16)
                if i >= 2:
                    # ACT must have drained g_ps/u_ps of i-2 (2 ACT ops per i)
                    nc.tensor.wait_ge(act_sem, 2 * (i - 1))
                for k in range(KT):
                    mm = nc.tensor.matmul(
                        g_ps[pp][:, :C],
                        w_sb[:, i * W + GW0 + k * 128 : i * W + GW0 + (k + 1) * 128],
                        x_sb[:, k * C : (k + 1) * C],
                        start=(k == 0),
                        stop=(k == KT - 1),
                    )
                mm.then_inc(pe_g, 1)
                for k in range(KT):
                    mm = nc.tensor.matmul(
                        u_ps[pp][:, :C],
                        w_sb[:, i * W + UW0 + k * 128 : i * W + UW0 + (k + 1) * 128],
                        x_sb[:, k * C : (k + 1) * C],
                        start=(k == 0),
                        stop=(k == KT - 1),
                    )
                mm.then_inc(pe_u, 1)
                nc.tensor.wait_ge(dve_sem, i + 1)  # a_T[i] ready
                for m in range(MT):
                    # start=True clears has_written for the WHOLE psum bank,
                    # so only the first (even) slice of each bank may set it;
                    # the odd slice's first write then lands on cleared
                    # has_written and overwrites cleanly.
                    mm = nc.tensor.matmul(
                        yslice(m),
                        w_sb[:, i * W + DW0 + m * 128 : i * W + DW0 + (m + 1) * 128],
                        a_sb[:, i * C : (i + 1) * C],
                        start=(i == 0 and m % 2 == 0),
                        stop=(i == IT - 1),
                        skip_group_check=True,
                    )
                if i == IT - 1:
                    mm.then_inc(pe_done, 1)

        @block.scalar
        def _(scalar):
            for i in range(IT):
                pp = i % 2
                nc.scalar.wait_ge(pe_g, i + 1)
                nc.scalar.activation(
                    sg_sb[:, i * C : (i + 1) * C],
                    g_ps[pp][:, :C],
                    mybir.ActivationFunctionType.Silu,
                ).then_inc(act_sem, 1)
                nc.scalar.wait_ge(pe_u, i + 1)
                nc.scalar.copy(
                    us_sb[:, i * C : (i + 1) * C], u_ps[pp][:, :C]
                ).then_inc(act_sem, 1)

        @block.vector
        def _(vector):
            for i in range(IT):
                nc.vector.wait_ge(act_sem, 2 * i + 2)
                nc.vector.tensor_mul(
                    a_sb[:, i * C : (i + 1) * C],
                    sg_sb[:, i * C : (i + 1) * C],
                    us_sb[:, i * C : (i + 1) * C],
                ).then_inc(dve_sem, 1)
            nc.vector.wait_ge(pe_done, 1)
            for m in range(MT):
                nc.vector.tensor_copy(
                    y_sb[:, m * C : (m + 1) * C], yslice(m)
                ).then_inc(dve_sem, 1)

    return nc


def _bf(x):
    return np.ascontiguousarray(x).astype(ml_dtypes.bfloat16)


def run(hidden_states, router_w, gate_w, up_w, down_w, trace=False):
    h = np.asarray(hidden_states, dtype=np.float32)
    rw = np.asarray(router_w, dtype=np.float32)
    gw = np.asarray(gate_w, dtype=np.float32)
    uw = np.asarray(up_w, dtype=np.float32)
    dw = np.asarray(down_w, dtype=np.float32)

    T = S * B
    hf = h.reshape(T, H)
    logits = hf.astype(np.float64) @ rw.astype(np.float64).T
    ids = logits.argmax(-1)
    idx = [np.where(ids == e)[0] for e in range(E)]
    maxc = max(len(s) for s in idx)
    C = max(128, -(-maxc // 32) * 32)

    if C not in _nc_cache:
        _nc_cache[C] = _build(C)
    nc = _nc_cache[C]

    in_maps = []
    for e in range(E):
        sel = idx[e]
        xp = np.zeros((C, H), np.float32)
        xp[: len(sel)] = hf[sel]
        # xt[p, k*C+c] = x[c, k*128+p]
        xt = _bf(xp.reshape(C, KT, 128).transpose(2, 1, 0).reshape(128, KT * C))
        # gwt[i, p, k*128+m] = gate_w[e][i*128+m, k*128+p]
        gwt = gw[e].reshape(IT, 128, KT, 128).transpose(0, 3, 2, 1).reshape(IT, 128, KT * 128)
        uwt = uw[e].reshape(IT, 128, KT, 128).transpose(0, 3, 2, 1).reshape(IT, 128, KT * 128)
        # dwt[i, p, m*128+mm] = down_w[e][m*128+mm, i*128+p]
        dwt = dw[e].reshape(MT, 128, IT, 128).transpose(2, 3, 0, 1).reshape(IT, 128, MT * 128)
        wtv = _bf(np.concatenate([gwt, uwt, dwt], axis=2))
        in_maps.append({"xt": xt, "wt": wtv})

    res = run_bass_kernel_spmd(nc, in_maps, core_ids=list(range(E)), trace=trace)

    out = np.zeros((T, H), np.float32)
    for e in range(E):
        ytv = np.asarray(res.results[e]["yt"], dtype=np.float32)
        # y[c, m*128+p] = yt[p, m*C+c]
        y = ytv.reshape(128, MT, C).transpose(2, 1, 0).reshape(C, H)
        out[idx[e]] = y[: len(idx[e])]
    return out.reshape(S, B, H), res


def kernel(**inputs) -> np.ndarray:
    out, _ = run(**inputs)
    return out



# revision 2
# speedup vs baseline: 1.5574x; 1.5574x over previous
"""Expert-parallel MoE (top-1 routing) kernel for 8 TRN2 NeuronCores.

Strategy (per the expert-parallel sharding hint): the 8 experts are sharded
1:1 across the 8 cores. The router is a 0.1%-of-FLOPs linear; it is computed
host-side in float64 to decide the token->expert dispatch (the all-to-all is
realized as the host->device sharding itself: each token's activations are
DMA'd only to the core owning its expert). Each core then runs the dense
expert MLP  y = (silu(x @ gw.T) * (x @ up.T)) @ dw.T  over its gathered
tokens (padded to a uniform capacity C) with fp32 PSUM accumulation.

Layout: everything on device is kept "activation-transposed" so all three
matmuls contract over the partition dimension with zero on-device transposes:
  g_T[i_tile] = sum_k gwT[k, i].T @ x_T[k]      (psum [128(I), C])
  a_T = silu(g_T) * u_T                          (sbuf bf16)
  y_T[m_tile] += dwT[i, m].T @ a_T[i]            (psum [128(H), C], 22-step acc)

Precision: gate/up weights are stored as fp8-e3m4 (power-of-two pre-scale,
descale folded exactly into the silu scale and the DVE multiply), halving
their HBM traffic; down weights and activations stay bf16. The PE runs
mixed-dtype matmuls (fp8 stationary, bf16 moving) at the bf16 rate.

Software pipeline (raw bass, per-engine streams): the down-projection
matmuls for i-tile i-1 are issued AFTER gate/up of i-tile i, so the PE never
stalls waiting for the ACT->DVE chain of the same iteration:
  SP     : x DMA, 11 paired gate/up weight DMAs, final single y DMA
  GpSimd : 11 paired down weight DMAs (SWDGE; otherwise-idle engine)
  PE     : per i: 8 g-matmuls, 8 u-matmuls, then 8 y-matmuls of i-1
  ACT    : per i: silu(g)->sbuf (with 1/Sg descale); tail: 4 psum->sbuf
  DVE    : per i: a_T[i] = silu_g * u' * (1/Su) (bf16); tail: 4 psum->sbuf
"""

import math

import numpy as np
import ml_dtypes
from contextlib import ExitStack

import concourse.bass as bass
import concourse.mybir as mybir
from concourse.alu_op_type import AluOpType
from concourse.bass_utils import run_bass_kernel_spmd

S, B, H, I, E = 512, 2, 1024, 2816, 8
KT, IT, MT = H // 128, I // 128, H // 128  # 8, 22, 8
JT = IT // 2  # 11 weight-DMA pairs
_BF = mybir.dt.bfloat16
_F8 = mybir.dt.float8e3  # e3m4
_F32 = mybir.dt.float32

GU_FP8 = True  # gate/up weights in fp8-e3m4 (halves their HBM bytes)

_nc_cache: dict = {}


def _build(C: int, inv_sg: float, inv_su: float) -> bass.Bass:
    """One-core program; SPMD across 8 cores (same shapes, per-core data)."""
    nc = bass.Bass()
    GUW = 2 * KT * 128  # gate|up cols per i-tile (2048)
    DW = MT * 128  # down cols per i-tile (1024)
    xt = nc.dram_tensor("xt", [128, KT * C], _BF, kind="ExternalInput")
    if GU_FP8:
        # per pair j: [i=2j: gate|up][i=2j+1: gate|up]
        w8t = nc.dram_tensor("w8t", [JT, 128, 2 * GUW], _F8, kind="ExternalInput")
        # per pair j: [i=2j: down][i=2j+1: down]
        wdt = nc.dram_tensor("wdt", [JT, 128, 2 * DW], _BF, kind="ExternalInput")
    else:
        # packed per i-tile: [gate (KT*128) | up (KT*128) | down (MT*128)]
        wt = nc.dram_tensor("wt", [IT, 128, GUW + DW], _BF, kind="ExternalInput")
    yt = nc.dram_tensor("yt", [128, MT * C], _F32, kind="ExternalOutput")

    assert C + 256 <= 512, "two y slices must fit one PSUM bank"

    with ExitStack() as ctx:
        x_sb = ctx.enter_context(nc.sbuf_tensor([128, KT * C], _BF))
        if GU_FP8:
            w8_sb = ctx.enter_context(nc.sbuf_tensor([128, IT * GUW], _F8))
            wd_sb = ctx.enter_context(nc.sbuf_tensor([128, IT * DW], _BF))
        else:
            w_sb = ctx.enter_context(nc.sbuf_tensor([128, IT * (GUW + DW)], _BF))
        sg_sb = ctx.enter_context(nc.sbuf_tensor([128, IT * C], _F32))
        a_sb = ctx.enter_context(nc.sbuf_tensor([128, IT * C], _BF))
        y_sb = ctx.enter_context(nc.sbuf_tensor([128, MT * C], _F32))
        # every PSUM tensor is one full 2 KiB bank ([128, 512] f32): matmul
        # outputs must not cross bank boundaries, and the bump allocator
        # would otherwise pack tensors across banks
        g_ps = [
            ctx.enter_context(nc.psum_tensor(f"g_ps{j}", [128, 512], _F32))
            for j in range(2)
        ]
        u_ps = [
            ctx.enter_context(nc.psum_tensor(f"u_ps{j}", [128, 512], _F32))
            for j in range(2)
        ]
        y_ps = [
            ctx.enter_context(nc.psum_tensor(f"y_ps{j}", [128, 512], _F32))
            for j in range(4)
        ]

        def yslice(m):
            return y_ps[m // 2][:, (m % 2) * 256 : (m % 2) * 256 + C]

        def gw_tile(i, k):
            if GU_FP8:
                base = i * GUW
                return w8_sb[:, base + k * 128 : base + (k + 1) * 128]
            base = i * (GUW + DW)
            return w_sb[:, base + k * 128 : base + (k + 1) * 128]

        def uw_tile(i, k):
            if GU_FP8:
                base = i * GUW + KT * 128
                return w8_sb[:, base + k * 128 : base + (k + 1) * 128]
            base = i * (GUW + DW) + KT * 128
            return w_sb[:, base + k * 128 : base + (k + 1) * 128]

        def dw_tile(i, m):
            if GU_FP8:
                base = i * DW
                return wd_sb[:, base + m * 128 : base + (m + 1) * 128]
            base = i * (GUW + DW) + GUW
            return w_sb[:, base + m * 128 : base + (m + 1) * 128]

        NW = JT if GU_FP8 else IT
        x_sem = ctx.enter_context(nc.semaphore())
        w_sem = [ctx.enter_context(nc.semaphore(name=f"w_sem{j}")) for j in range(NW)]
        if GU_FP8:
            wd_sem = [
                ctx.enter_context(nc.semaphore(name=f"wd_sem{j}")) for j in range(JT)
            ]
        pe_g = ctx.enter_context(nc.semaphore())
        pe_u = ctx.enter_context(nc.semaphore())
        pe_done = ctx.enter_context(nc.semaphore())
        act_sem = ctx.enter_context(nc.semaphore())
        dve_sem = ctx.enter_context(nc.semaphore())
        dma_sem = ctx.enter_context(nc.semaphore())
        block = ctx.enter_context(nc.Block())

        @block.sync
        def _(sync):
            nc.sync.dma_start(x_sb[:], xt[:]).then_inc(x_sem, 16)
            if GU_FP8:
                for j in range(JT):
                    nc.sync.dma_start(
                        w8_sb[:, j * 2 * GUW : (j + 1) * 2 * GUW], w8t[j]
                    ).then_inc(w_sem[j], 16)
            else:
                W = GUW + DW
                for i in range(IT):
                    nc.sync.dma_start(
                        w_sb[:, i * W : (i + 1) * W], wt[i]
                    ).then_inc(w_sem[i], 16)
            nc.sync.wait_ge(act_sem, IT + 4)
            nc.sync.wait_ge(dve_sem, IT + 4)
            nc.sync.dma_start(yt[:], y_sb[:]).then_inc(dma_sem, 16)
            nc.sync.wait_ge(dma_sem, 16)

        if GU_FP8:

            @block.gpsimd
            def _(gpsimd):
                for j in range(JT):
                    nc.gpsimd.dma_start(
                        wd_sb[:, j * 2 * DW : (j + 1) * 2 * DW], wdt[j]
                    ).then_inc(wd_sem[j], 16)

        def y_block(tensor, i, stop):
            for m in range(MT):
                # start=True clears has_written for the WHOLE psum bank,
                # so only the first (even) slice of each bank may set it;
                # the odd slice's first write then lands on cleared
                # has_written and overwrites cleanly.
                mm = nc.tensor.matmul(
                    yslice(m),
                    dw_tile(i, m),
                    a_sb[:, i * C : (i + 1) * C],
                    start=(i == 0 and m % 2 == 0),
                    stop=stop,
                    skip_group_check=True,
                )
            return mm

        @block.tensor
        def _(tensor):
            nc.tensor.wait_ge(x_sem, 16)
            for i in range(IT):
                pp = i % 2
                if GU_FP8:
                    if i % 2 == 0:
                        nc.tensor.wait_ge(w_sem[i // 2], 16)
                else:
                    nc.tensor.wait_ge(w_sem[i], 16)
                if i >= 2:
                    # g/u psum bank pp free once DVE's mul of i-2 is done
                    nc.tensor.wait_ge(dve_sem, i - 1)
                for k in range(KT):
                    mm = nc.tensor.matmul(
                        g_ps[pp][:, :C],
                        gw_tile(i, k),
                        x_sb[:, k * C : (k + 1) * C],
                        start=(k == 0),
                        stop=(k == KT - 1),
                    )
                mm.then_inc(pe_g, 1)
                for k in range(KT):
                    mm = nc.tensor.matmul(
                        u_ps[pp][:, :C],
                        uw_tile(i, k),
                        x_sb[:, k * C : (k + 1) * C],
                        start=(k == 0),
                        stop=(k == KT - 1),
                    )
                mm.then_inc(pe_u, 1)
                if i >= 1:
                    # y-matmuls run one iteration behind: a_T[i-1] was
                    # produced while g/u of i were on the PE, so no stall
                    nc.tensor.wait_ge(dve_sem, i)
                    if GU_FP8 and (i - 1) % 2 == 0:
                        nc.tensor.wait_ge(wd_sem[(i - 1) // 2], 16)
                    y_block(tensor, i - 1, stop=False)
            nc.tensor.wait_ge(dve_sem, IT)
            y_block(tensor, IT - 1, stop=True).then_inc(pe_done, 1)

        @block.scalar
        def _(scalar):
            for i in range(IT):
                pp = i % 2
                nc.scalar.wait_ge(pe_g, i + 1)
                nc.scalar.activation(
                    sg_sb[:, i * C : (i + 1) * C],
                    g_ps[pp][:, :C],
                    mybir.ActivationFunctionType.Silu,
                    scale=inv_sg,
                ).then_inc(act_sem, 1)
            nc.scalar.wait_ge(pe_done, 1)
            for m in range(4, MT):
                nc.scalar.copy(
                    y_sb[:, m * C : (m + 1) * C], yslice(m)
                ).then_inc(act_sem, 1)

        @block.vector
        def _(vector):
            for i in range(IT):
                pp = i % 2
                nc.vector.wait_ge(act_sem, i + 1)
                nc.vector.wait_ge(pe_u, i + 1)
                if GU_FP8:
                    # a = (u' * 1/Su) * silu_g   (1/Su is a power of two)
                    nc.vector.scalar_tensor_tensor(
                        a_sb[:, i * C : (i + 1) * C],
                        u_ps[pp][:, :C],
                        inv_su,
                        sg_sb[:, i * C : (i + 1) * C],
                        AluOpType.mult,
                        AluOpType.mult,
                    ).then_inc(dve_sem, 1)
                else:
                    nc.vector.tensor_mul(
                        a_sb[:, i * C : (i + 1) * C],
                        sg_sb[:, i * C : (i + 1) * C],
                        u_ps[pp][:, :C],
                    ).then_inc(dve_sem, 1)
            nc.vector.wait_ge(pe_done, 1)
            for m in range(4):
                nc.vector.tensor_copy(
                    y_sb[:, m * C : (m + 1) * C], yslice(m)
                ).then_inc(dve_sem, 1)

    return nc


def _bf(x):
    return np.ascontiguousarray(x).astype(ml_dtypes.bfloat16)


def _pow2_scale(absmax: float, dt) -> float:
    fmax = float(ml_dtypes.finfo(dt).max)
    return 2.0 ** math.floor(math.log2((fmax * 0.5) / absmax))


def run(hidden_states, router_w, gate_w, up_w, down_w, trace=False):
    h = np.asarray(hidden_states, dtype=np.float32)
    rw = np.asarray(router_w, dtype=np.float32)
    gw = np.asarray(gate_w, dtype=np.float32)
    uw = np.asarray(up_w, dtype=np.float32)
    dw = np.asarray(down_w, dtype=np.float32)

    T = S * B
    hf = h.reshape(T, H)
    logits = hf.astype(np.float64) @ rw.astype(np.float64).T
    ids = logits.argmax(-1)
    idx = [np.where(ids == e)[0] for e in range(E)]
    maxc = max(len(s) for s in idx)
    C = max(128, -(-maxc // 8) * 8)

    if GU_FP8:
        sg = _pow2_scale(float(np.abs(gw).max()), ml_dtypes.float8_e3m4)
        su = _pow2_scale(float(np.abs(uw).max()), ml_dtypes.float8_e3m4)
    else:
        sg = su = 1.0

    key = (C, sg, su)
    if key not in _nc_cache:
        _nc_cache[key] = _build(C, 1.0 / sg, 1.0 / su)
    nc = _nc_cache[key]

    in_maps = []
    for e in range(E):
        sel = idx[e]
        xp = np.zeros((C, H), np.float32)
        xp[: len(sel)] = hf[sel]
        # xt[p, k*C+c] = x[c, k*128+p]
        xt = _bf(xp.reshape(C, KT, 128).transpose(2, 1, 0).reshape(128, KT * C))
        # gwt[i, p, k*128+m] = gate_w[e][i*128+m, k*128+p]
        gwt = gw[e].reshape(IT, 128, KT, 128).transpose(0, 3, 2, 1).reshape(IT, 128, KT * 128)
        uwt = uw[e].reshape(IT, 128, KT, 128).transpose(0, 3, 2, 1).reshape(IT, 128, KT * 128)
        # dwt[i, p, m*128+mm] = down_w[e][m*128+mm, i*128+p]
        dwt = dw[e].reshape(MT, 128, IT, 128).transpose(2, 3, 0, 1).reshape(IT, 128, MT * 128)
        if GU_FP8:
            gu = np.concatenate([gwt * sg, uwt * su], axis=2)  # [IT,128,2*KT*128]
            w8 = np.ascontiguousarray(
                gu.reshape(JT, 2, 128, 2 * KT * 128).transpose(0, 2, 1, 3)
                .reshape(JT, 128, 4 * KT * 128)
            ).astype(ml_dtypes.float8_e3m4)
            wdv = _bf(
                dwt.reshape(JT, 2, 128, MT * 128).transpose(0, 2, 1, 3)
                .reshape(JT, 128, 2 * MT * 128)
            )
            in_maps.append({"xt": xt, "w8t": w8, "wdt": wdv})
        else:
            wtv = _bf(np.concatenate([gwt, uwt, dwt], axis=2))
            in_maps.append({"xt": xt, "wt": wtv})

    res = run_bass_kernel_spmd(nc, in_maps, core_ids=list(range(E)), trace=trace)

    out = np.zeros((T, H), np.float32)
    for e in range(E):
        ytv = np.asarray(res.results[e]["yt"], dtype=np.float32)
        # y[c, m*128+p] = yt[p, m*C+c]
        y = ytv.reshape(128, MT, C).transpose(2, 1, 0).reshape(C, H)
        out[idx[e]] = y[: len(idx[e])]
    return out.reshape(S, B, H), res


def kernel(**inputs) -> np.ndarray:
    out, _ = run(**inputs)
    return out
